# revision 63
# baseline (speedup 1.0000x reference)
"""Trainium2 Bass kernel for Llama-style GQA attention block.

Contract: kernel(**inputs) takes FULL unsharded inputs (as produced by the
problem's setup_inputs) and returns the FULL [B, S, H] output.

Sharding: tensor-parallel over heads across 8 NeuronCores. Core i computes
q-heads 4i..4i+3 and kv-head i (GQA group i), produces a partial o_proj
output [B, S, H]; partials are summed on the host (the all-reduce).

Shapes (hardcoded): B=2, S=2048, H=4096, NH=32, NKV=8, HD=128.
"""

import sys

for _p in ("/opt/trn_rl_repo",):
    if _p not in sys.path:
        sys.path.insert(0, _p)

import numpy as np

import concourse.bacc as bacc
import concourse.mybir as mybir
import concourse.tile as tile
from concourse.bass_utils import run_bass_kernel_spmd
from concourse.masks import make_identity

B, S, H = 2, 2048, 4096
NH, NKV, HD = 32, 8, 128
ROPE_THETA = 10000.0
NCORES = 8
QH = NH // NCORES            # 4 q heads per core
SB = S // 512                # 4 s-blocks of 512 per batch
HC = H // 128                # 32 h-chunks
KVC = S // 128               # 16 kv chunks
EXP_BIAS = -5.0              # exp(score - 5): keeps P in fp16 range

f32 = mybir.dt.float32
f32r = mybir.dt.float32r
f16 = mybir.dt.float16

_NC_CACHE: dict = {}


def _r(ap):
    return ap.bitcast(f32r)


def _build(mode: str, reps: int = 1):
    """mode: 'none' (no mask), 'causal', or 'general' (mask streamed)."""
    nc = bacc.Bacc("TRN2", target_bir_lowering=False, debug=False,
                   num_devices=NCORES)

    xt = nc.dram_tensor("xt", [H, B * S], f32, kind="ExternalInput").ap()
    wqkv = nc.dram_tensor("wqkv", [128, HC * 768], f32,
                          kind="ExternalInput").ap()
    wo = nc.dram_tensor("wo", [QH * HD, H], f32, kind="ExternalInput").ap()
    trig = nc.dram_tensor("trig", [B, 2, 128, S], f32,
                          kind="ExternalInput").ap()
    if mode == "causal":
        mdiag = nc.dram_tensor("mdiag", [4, 128, 512], f32,
                               kind="ExternalInput").ap()
    elif mode == "general":
        maskt = nc.dram_tensor("maskt", [B, S, S], f32,
                               kind="ExternalInput").ap()
    out = nc.dram_tensor("out", [B, S, H], f32, kind="ExternalOutput").ap()

    wqkv3 = wqkv.rearrange("p (c o) -> p c o", o=768)

    with tile.TileContext(nc) as tc:
        # ---- persistent pools (whole kernel) ----
        with tc.tile_pool(name="perm", bufs=1) as perm, \
             tc.tile_pool(name="dram", bufs=1, space="DRAM") as dpool:
            kt_sb = [perm.tile([128, S], f32r, tag=f"kt{b}", name=f"kt{b}") for b in range(B)]
            vt_sb = [perm.tile([128, S], f16, tag=f"vt{b}", name=f"vt{b}") for b in range(B)]
            vaug = [perm.tile([128, KVC, 130], f16, tag=f"va{b}", name=f"va{b}")
                    for b in range(B)]
            ident = perm.tile([128, 128], f16, tag="ident", name="ident")
            make_identity(nc, ident[:])
            bias_t = perm.tile([128, 1], f32, tag="bias", name="bias_t")
            nc.gpsimd.memset(bias_t[:], EXP_BIAS)
            qt_dram = dpool.tile([B, QH, 128, S], f32, name="qt_dram")

            for _rep in range(reps):
                # ================= Phase A: projections + RoPE =================
                with tc.tile_pool(name="wq", bufs=1) as wq_pool, \
                     tc.tile_pool(name="cs", bufs=4) as cs_pool, \
                     tc.tile_pool(name="xtp", bufs=4) as xt_pool, \
                     tc.tile_pool(name="rope", bufs=6) as rope_pool, \
                     tc.tile_pool(name="psA", bufs=8, space="PSUM") as psA:

                    w_sb = wq_pool.tile([128, HC, 768], f32r, tag="w", name="w_sb")
                    for _wc in range(HC):
                        nc.sync.dma_start(w_sb[:, _wc, :],
                                          wqkv3[:, _wc, :].bitcast(f32r))

                    for b in range(B):
                        for sb in range(SB):
                            ssl = slice(sb * 512, sb * 512 + 512)
                            cos_t = cs_pool.tile([128, 512], f32, tag="cos", name="cos_t")
                            sin_t = cs_pool.tile([128, 512], f32, tag="sin", name="sin_t")
                            nc.sync.dma_start(cos_t[:], trig[b, 0, :, ssl])
                            nc.sync.dma_start(sin_t[:], trig[b, 1, :, ssl])

                            psums = [psA.tile([128, 512], f32, tag="pA", name=f"pA{_j}")
                                     for _j in range(6)]
                            for hc in range(HC):
                                xt_t = xt_pool.tile([128, 512], f32r, tag="xt", name="xt_t")
                                nc.sync.dma_start(
                                    xt_t[:],
                                    xt[hc * 128:(hc + 1) * 128,
                                       b * S + sb * 512:
                                       b * S + sb * 512 + 512].bitcast(f32r))
                                for j in range(6):
                                    o0 = j * 128
                                    nc.tensor.matmul(
                                        psums[j][:],
                                        w_sb[:, hc, o0:o0 + 128],
                                        xt_t[:],
                                        start=(hc == 0), stop=(hc == HC - 1),
                                        skip_group_check=True)

                            # free PSUM banks fast: ScalarE copies to SBUF
                            # staging; v cast directly. RoPE then runs on DVE
                            # from SBUF, overlapped with next block's matmuls.
                            stages = []
                            for j in range(QH + 1):
                                stg_p = rope_pool.tile([128, 512], f32,
                                                       tag="stgp", name="stg_p")
                                nc.scalar.copy(stg_p[:], psums[j][:])
                                stages.append(stg_p)
                            nc.scalar.copy(vt_sb[b][:, ssl], psums[5][:])
                            for j in range(QH + 1):
                                qc_t = rope_pool.tile([128, 512], f32, tag="rA", name="qc_t")
                                rot = rope_pool.tile([128, 512], f32, tag="rB", name="rot")
                                src_t = stages[j]
                                nc.vector.tensor_mul(qc_t[:], src_t[:],
                                                     cos_t[:])
                                nc.vector.tensor_mul(rot[0:64, :],
                                                     src_t[64:128, :],
                                                     sin_t[64:128, :])
                                nc.vector.tensor_mul(rot[64:128, :],
                                                     src_t[0:64, :],
                                                     sin_t[0:64, :])
                                if j < QH:
                                    stg = rope_pool.tile([128, 512], f32,
                                                         tag="rC", name="stg")
                                    nc.vector.tensor_add(stg[:], qc_t[:],
                                                         rot[:])
                                    nc.sync.dma_start(qt_dram[b, j, :, ssl],
                                                      stg[:])
                                else:
                                    nc.vector.tensor_add(kt_sb[b][:, ssl],
                                                         qc_t[:], rot[:])

                # ================= Phase B: attention + o_proj =================
                with tc.tile_pool(name="qtp", bufs=3) as qt_pool, \
                     tc.tile_pool(name="expp", bufs=17) as exp_pool, \
                     tc.tile_pool(name="outt", bufs=5) as outt_pool, \
                     tc.tile_pool(name="attn", bufs=4) as attn_pool, \
                     tc.tile_pool(name="invp", bufs=4) as inv_pool, \
                     tc.tile_pool(name="wop", bufs=8) as wo_pool, \
                     tc.tile_pool(name="otp", bufs=4) as ot_pool, \
                     tc.tile_pool(name="mskp", bufs=(1 if mode == "causal" else 4)) as msk_pool, \
                     tc.tile_pool(name="psB", bufs=2, space="PSUM") as psB, \
                     tc.tile_pool(name="psAV", bufs=2, space="PSUM") as psAV, \
                     tc.tile_pool(name="psT", bufs=2, space="PSUM") as psT, \
                     tc.tile_pool(name="psO", bufs=2, space="PSUM") as psO:

                    msk_sb = None
                    if mode == "causal":
                        msk_sb = [msk_pool.tile([128, 512], f32, tag=f"m{o}", name=f"m{o}")
                                  for o in range(4)]
                        for o in range(4):
                            nc.sync.dma_start(msk_sb[o][:], mdiag[o])

                    # V: [d, s] -> [s, d] via PE transpose, plus ones column
                    for b in range(B):
                        nc.vector.memset(vaug[b][:, :, 128:130], 0.0)
                        nc.vector.memset(vaug[b][:, :, 128:129], 1.0)
                        for ck in range(KVC):
                            ps_t = psT.tile([128, 128], f16, tag="pst", name="ps_t")
                            nc.tensor.transpose(
                                ps_t[:], vt_sb[b][:, ck * 128:(ck + 1) * 128],
                                ident[:])
                            nc.vector.tensor_copy(vaug[b][:, ck, 0:128], ps_t[:])

                    for b in range(B):
                        outt = [outt_pool.tile([128, S], f32r, tag="outt", name=f"outt{_h}")
                                for _h in range(QH)]
                        for h in range(QH):
                            for qb in range(SB):
                                qsl = slice(qb * 512, qb * 512 + 512)
                                qt_t = qt_pool.tile([128, 512], f32r, tag="qt", name="qt_t")
                                nc.sync.dma_start(qt_t[:],
                                                  qt_dram[b, h, :, qsl]
                                                  .bitcast(f32r))
                                if mode == "causal":
                                    kv_list = list(range(qb * 4 + 4))
                                else:
                                    kv_list = list(range(KVC))
                                exp_tiles = []
                                for kv in kv_list:
                                    ps = psB.tile([128, 512], f32, tag="psb", name="ps")
                                    nc.tensor.matmul(
                                        ps[:],
                                        kt_sb[b][:, kv * 128:(kv + 1) * 128],
                                        qt_t[:],
                                        start=True, stop=True,
                                        skip_group_check=True)
                                    if mode == "causal" and kv >= qb * 4:
                                        nc.vector.tensor_add(
                                            ps[:], ps[:], msk_sb[kv - qb * 4][:])
                                    elif mode == "general":
                                        mt = msk_pool.tile([128, 512], f32,
                                                           tag="mt", name="mt")
                                        nc.sync.dma_start(
                                            mt[:],
                                            maskt[b, kv * 128:(kv + 1) * 128,
                                                  qsl])
                                        nc.vector.tensor_add(ps[:], ps[:], mt[:])
                                    et = exp_pool.tile([128, 512], f16, tag="e", name="et")
                                    nc.scalar.activation(
                                        et[:], ps[:],
                                        mybir.ActivationFunctionType.Exp,
                                        bias=bias_t[:])
                                    exp_tiles.append((kv, et))
                                for qc in range(4):
                                    csl = slice(qc * 128, qc * 128 + 128)
                                    pav = psAV.tile([128, 132], f32, tag="pav", name="pav")
                                    n_e = len(exp_tiles)
                                    for idx, (kv, et) in enumerate(exp_tiles):
                                        nc.tensor.matmul(
                                            pav[:, 0:129],
                                            et[:, csl],
                                            vaug[b][:, kv, 0:129],
                                            start=(idx == 0),
                                            stop=(idx == n_e - 1),
                                            skip_group_check=True)
                                    inv = inv_pool.tile([128, 1], f32, tag="inv", name="inv")
                                    nc.vector.reciprocal(inv[:], pav[:, 128:129])
                                    at_t = attn_pool.tile([128, 128], f16,
                                                          tag="at", name="at_t")
                                    nc.vector.tensor_mul(
                                        at_t[:], pav[:, 0:128],
                                        inv[:].to_broadcast((128, 128)))
                                    ps_t = psT.tile([128, 128], f16, tag="pst", name="ps_t")
                                    nc.tensor.transpose(ps_t[:], at_t[:],
                                                        ident[:])
                                    nc.vector.tensor_copy(
                                        outt[h][:, (qb * 4 + qc) * 128:
                                                (qb * 4 + qc) * 128 + 128],
                                        ps_t[:])

                        # ---- o_proj partial for this batch ----
                        for mb in range(8):
                            msl = slice(mb * 512, mb * 512 + 512)
                            wo_ts = []
                            for h in range(QH):
                                wt = wo_pool.tile([128, 512], f32r, tag="wo", name="wt")
                                nc.sync.dma_start(
                                    wt[:],
                                    wo[h * 128:(h + 1) * 128, msl].bitcast(f32r))
                                wo_ts.append(wt)
                            for sc in range(16):
                                scl = slice(sc * 128, sc * 128 + 128)
                                po = psO.tile([128, 512], f32, tag="po", name="po")
                                for h in range(QH):
                                    nc.tensor.matmul(
                                        po[:], outt[h][:, scl],
                                        wo_ts[h][:],
                                        start=(h == 0), stop=(h == QH - 1),
                                        skip_group_check=True)
                                ot = ot_pool.tile([128, 512], f32, tag="ot", name="ot")
                                nc.vector.tensor_copy(ot[:], po[:])
                                nc.sync.dma_start(out[b, scl, msl], ot[:])

    nc.compile()
    return nc


def _host_prep(hidden_states, position_ids, Wq, Wk, Wv, Wo):
    """Per-core input maps. Core i: q heads QH*i..QH*i+QH-1, kv head i."""
    hs = np.asarray(hidden_states, dtype=np.float32)
    # Xt: [H, B*S], column order b-major
    xtr = np.ascontiguousarray(hs.reshape(B * S, H).T)

    # rope tables (match reference: float32 math)
    inv_freq = (1.0 / (ROPE_THETA **
                       (np.arange(0, HD, 2, dtype=np.float32) / HD))
                ).astype(np.float32)
    t = np.arange(S, dtype=np.float32)
    freqs = np.outer(t, inv_freq).astype(np.float32)       # [S, 64]
    emb = np.concatenate([freqs, freqs], axis=-1)          # [S, 128]
    cos_tab = np.cos(emb).astype(np.float32)
    sin_tab = np.sin(emb).astype(np.float32)
    pos = np.asarray(position_ids).astype(np.int64)        # [B, S]
    trig = np.empty((B, 2, 128, S), dtype=np.float32)
    for b in range(B):
        cb = cos_tab[pos[b]]                               # [S, 128]
        sb = sin_tab[pos[b]]
        sb2 = np.concatenate([sb[:, 0:64], -sb[:, 64:128]], axis=1)
        trig[b, 0] = cb.T
        trig[b, 1] = sb2.T

    Wq = np.asarray(Wq, dtype=np.float32)
    Wk = np.asarray(Wk, dtype=np.float32)
    Wv = np.asarray(Wv, dtype=np.float32)
    Wo = np.asarray(Wo, dtype=np.float32)
    scale = np.float32(1.0 / np.sqrt(HD))

    in_maps = []
    for i in range(NCORES):
        wq_i = (Wq[i * QH * HD:(i + 1) * QH * HD, :] * scale).T  # [H, 512]
        wk_i = Wk[i * HD:(i + 1) * HD, :].T                      # [H, 128]
        wv_i = Wv[i * HD:(i + 1) * HD, :].T
        cat = np.concatenate(
            [wq_i.reshape(HC, 128, QH * 128),
             wk_i.reshape(HC, 128, 128),
             wv_i.reshape(HC, 128, 128)], axis=2)                # [32,128,768]
        wqkv = np.ascontiguousarray(
            cat.transpose(1, 0, 2).reshape(128, HC * 768))
        wo_i = np.ascontiguousarray(
            Wo[:, i * QH * HD:(i + 1) * QH * HD].T)              # [512, H]
        in_maps.append({
            "xt": xtr, "wqkv": wqkv.astype(np.float32),
            "wo": wo_i.astype(np.float32), "trig": trig,
        })
    return in_maps


def _detect_mask_mode(attention_mask):
    m = np.asarray(attention_mask)
    if not np.any(m):
        return "none"
    # causal: per batch, upper-strict very negative, lower-inclusive zero
    tri = np.triu(np.ones((S, S), dtype=bool), k=1)
    for b in range(m.shape[0]):
        mb = m[b, 0]
        if not (np.all(mb[~tri] == 0.0) and np.all(mb[tri] <= -1e30)):
            return "general"
    return "causal"


def _causal_diag_tiles():
    """mdiag[o][kvr, qc]: additive mask for kv row (o*128+kvr) vs q col qc
    within a 512-wide q block: 0 if qc >= o*128+kvr else -inf."""
    neg = np.float32(np.finfo(np.float32).min)
    md = np.zeros((4, 128, 512), dtype=np.float32)
    qc = np.arange(512)[None, :]
    for o in range(4):
        kvr = o * 128 + np.arange(128)[:, None]
        md[o] = np.where(qc >= kvr, 0.0, neg)
    return md


def kernel(hidden_states, attention_mask, position_ids, Wq, Wk, Wv, Wo):
    mode = _detect_mask_mode(attention_mask)
    if mode not in _NC_CACHE:
        _NC_CACHE[mode] = _build(mode)
    nc = _NC_CACHE[mode]

    in_maps = _host_prep(hidden_states, position_ids, Wq, Wk, Wv, Wo)
    if mode == "causal":
        md = _causal_diag_tiles()
        for im in in_maps:
            im["mdiag"] = md
    elif mode == "general":
        mt = np.ascontiguousarray(
            np.asarray(attention_mask, dtype=np.float32)[:, 0]
            .transpose(0, 2, 1))
        for im in in_maps:
            im["maskt"] = mt

    res = run_bass_kernel_spmd(nc, in_maps, core_ids=list(range(NCORES)))
    parts = [res.results[i]["out"] for i in range(NCORES)]
    out = parts[0]
    for p in parts[1:]:
        out = out + p
    return out.astype(np.float32)
